# revision 1
# baseline (speedup 1.0000x reference)
"""Trainium2 Bass kernel for nn_AlwGAT (GAT-style message passing), v2.

Math (equivalent to the reference):
  self = x[:, :36]; others = x[:, 36:].reshape(B, 19, 28)
  att  = softmax_j(others_j . Wa[36:])          # self-part cancels (shift inv.)
  out  = self @ A_self + (sum_j att_j * others_j) @ A_pool + c
where
  A_self = We[:36] @ Wo[:64] + (Ws[:36] + Ws[36:]) @ Wo[64:]
  A_pool = We[36:] @ Wo[:64]
  c      = be @ Wo[:64] + bs @ Wo[64:] + bo     (added on host; zeros here)

Dataflow (per 256-row tile, 2 half-blocks of 128 rows):
  host: x cast to bf16 and padded 568->640; row r0+16p+s lives on DRAM so
     that partition p gets rows 16p..16p+15 -> 20KB contiguous per-partition
     DMA runs (line rate), and the same (p s) mapping makes the output DMA
     contiguous too.
  ACT-queue HWDGE load: xbig [128, 16, 640] bf16 per 8-tile group
  XBAR DMA-transpose (sync queue, SBUF->SBUF): per 2 tiles,
     xt[p, m, r] = xbig[r, m//5, 128*(m%5)+p] -- feature chunks on
     partitions; junk pad partitions in chunk 4 are never read
  PE logits: lT[19, 2*128] = sum_c WL_c^T @ xt_c  (10 mm, h-outer so the
     two PSUM accumulation groups in one bank stay contiguous)
  ACT exp -> eT bf16; PE ones-matmul -> s per row; DVE recip -> rr
  PE erep: e broadcast to features via 0/1 selector, c-major 256-col mms
     (ones rows for self features make erep_self = s, folding the softmax
     denominator)
  DVE: sp = xt * erep (bf16)
  PE final: out[128, 2, 64] = sum_c sp_c^T @ FW_c   (data-stationary, ap=64)
  ACT: out_sbuf = out_psum * (1/s)  (Copy activation, per-partition scale)
  sync DMA out per 8-tile group.
"""

import os
import sys

if "/opt/trn_rl_repo" not in sys.path:
    sys.path.insert(0, "/opt/trn_rl_repo")

import numpy as np

SELF = 36
OTH = 28
J = 19
H = 64
OBS = SELF + OTH * J  # 568
NCORES = 8
BATCH = 65536
ROWS_PER_CORE = BATCH // NCORES  # 8192
TILE_ROWS = 256
NT = ROWS_PER_CORE // TILE_ROWS  # 32
F = [128, 128, 128, 128, 56]  # real features per chunk (5 x 128 covers 640)
FFIN = [128, 128, 128, 128, 57]  # final contraction includes the s-row (p=56,c=4)
H2 = H + 2  # final out cols: 64 outputs + s + pad
NCH = 5
PADF = 640  # 5*128, XBAR needs free %128
GRP = 8  # tiles per load/store group
NG = NT // GRP

_CACHE = {}


def _build_nc():
    import concourse.bass as bass  # noqa: F401
    import concourse.tile as tile
    from concourse import bacc, mybir

    f32 = mybir.dt.float32
    bf16 = mybir.dt.bfloat16

    nc = bacc.Bacc("TRN2", debug=False)
    x_d = nc.dram_tensor(
        "x_in", [ROWS_PER_CORE, PADF], bf16, kind="ExternalInput"
    ).ap()
    wl_d = nc.dram_tensor("wl_in", [128, NCH, J + 1], f32, kind="ExternalInput").ap()
    b_d = nc.dram_tensor("bsel_in", [J, NCH, 128], f32, kind="ExternalInput").ap()
    fw_d = nc.dram_tensor("fw_in", [128, NCH, H2], f32, kind="ExternalInput").ap()
    out_d = nc.dram_tensor("out", [ROWS_PER_CORE, H], f32, kind="ExternalOutput").ap()

    Exp = mybir.ActivationFunctionType.Exp
    Copy = mybir.ActivationFunctionType.Copy

    with tile.TileContext(nc) as tc:
        with (
            tc.tile_pool(name="consts", bufs=1) as consts,
            tc.tile_pool(name="xbig", bufs=2) as xbig_pool,
            tc.tile_pool(name="xt", bufs=4) as xt_pool,
            tc.tile_pool(name="eT", bufs=2) as eT_pool,
            tc.tile_pool(name="rr", bufs=2) as r_pool,
            tc.tile_pool(name="sp", bufs=2) as sp_pool,
            tc.tile_pool(name="obig", bufs=2) as obig_pool,
            tc.tile_pool(name="psLT", bufs=2, space="PSUM") as lt_pool,
            tc.tile_pool(name="psER", bufs=1, space="PSUM") as erep_pool,
            tc.tile_pool(name="psOUT", bufs=2, space="PSUM") as ops_pool,
        ):
            # constants: stage f32, convert once to bf16
            wl_st = consts.tile([128, NCH, J + 1], f32)
            nc.sync.dma_start(out=wl_st, in_=wl_d)
            wl_sb = consts.tile([128, NCH, J + 1], bf16)
            nc.scalar.copy(out=wl_sb, in_=wl_st)
            b_st = consts.tile([J, NCH, 128], f32)
            nc.sync.dma_start(out=b_st, in_=b_d)
            b_sb = consts.tile([J, NCH, 128], bf16)
            nc.scalar.copy(out=b_sb, in_=b_st)
            fw_st = consts.tile([128, NCH, H2], f32)
            nc.sync.dma_start(out=fw_st, in_=fw_d)
            fw_sb = consts.tile([128, NCH, H2], bf16)
            nc.scalar.copy(out=fw_sb, in_=fw_st)

            st = {}

            def do_load(g):
                # x arrives bf16 640-padded from the host: row r0+16p+s lives
                # on partition p, slot s -> one 20 KB contiguous run per
                # partition (line-rate DMA, no on-device cast needed).  The
                # load goes on the gpsimd SWDGE queue so it cannot serialize
                # against the XBAR transposes on the sync HWDGE ring.
                r0 = g * GRP * TILE_ROWS
                xb = xbig_pool.tile([128, 2 * GRP, PADF], bf16, tag="xbig")
                nc.scalar.dma_start(
                    out=xb,
                    in_=x_d[r0 : r0 + GRP * TILE_ROWS, :].rearrange(
                        "(p s) f -> p s f", p=128
                    ),
                )
                st[("xb", g)] = xb
                ob = obig_pool.tile([128, 2 * GRP, H], f32, tag="obig")
                st[("ob", g)] = ob


            def s_load(t):
                # prefetch one group ahead (groups 0 and 1 primed pre-loop)
                if t % GRP:
                    return
                g = t // GRP + 2
                if g < NG:
                    do_load(g)

            def s_xbar(t):
                if t % 2:
                    return
                xb = st[("xb", t // GRP)]
                hh = (t % GRP) * 2
                xt = xt_pool.tile([128, 4 * NCH, 128], bf16, tag="xt")
                nc.sync.dma_start(out=xt, in_=xb[:, hh : hh + 4, :], transpose=True)
                st[("xt", t // 2)] = xt

            def s_logits(t):
                xt = st[("xt", t // 2)]
                mo = (t % 2) * 2 * NCH
                lt = lt_pool.tile([128, 512], f32, tag="lt")
                # h-outer: keep each PSUM accumulation group contiguous
                for h in range(2):
                    for c in range(NCH):
                        fc = F[c]
                        nc.tensor.matmul(
                            lt[0:J, 128 * h : 128 * (h + 1)],
                            wl_sb[0:fc, c, 0:J],
                            xt[0:fc, mo + NCH * h + c, :],
                            start=(c == 0),
                            stop=(c == NCH - 1),
                        )
                st[("lt", t)] = lt

            def s_att(t):
                lt = st[("lt", t)]
                eT = eT_pool.tile([J, 256], bf16, tag="eT")
                nc.scalar.activation(out=eT, in_=lt[0:J, 0:256], func=Exp)
                st[("eT", t)] = eT
                del st[("lt", t)]

            def s_erep(t):
                eT = st[("eT", t)]
                # c-major layout: one 256-col matmul per chunk (both halves),
                # halving the 128-col LDWEIGHTS count
                er = erep_pool.tile([128, NCH, 2, 128], f32, tag="er")
                for c in range(NCH):
                    nc.tensor.matmul(
                        er[:, c, :, :],
                        b_sb[:, c, :],
                        eT,
                        start=True,
                        stop=True,
                    )
                st[("er", t)] = er
                del st[("eT", t)]

            def s_sp(t):
                xt = st[("xt", t // 2)]
                mo = (t % 2) * 2 * NCH
                er = st[("er", t)]
                sp = sp_pool.tile([128, 2, NCH, 128], bf16, tag="sp")
                # DVE muls per half (gpsimd cannot read PSUM); er is c-major
                for h in range(2):
                    nc.vector.tensor_mul(
                        sp[:, h, :, :],
                        xt[:, mo + NCH * h : mo + NCH * (h + 1), :],
                        er[:, :, h, :],
                    )
                st[("sp", t)] = sp
                del st[("er", t)]

            def s_final(t):
                sp = st.pop(("sp", t))
                ob = st[("ob", t // GRP)]
                ops = ops_pool.tile([128, 2, H2], f32, tag="ops")
                rr = r_pool.tile([128, 2], f32, tag="rr")
                for h in range(2):
                    for c in range(NCH):
                        fc = FFIN[c]
                        nc.tensor.matmul(
                            ops[:, h, :],
                            sp[0:fc, h, c, :],
                            fw_sb[0:fc, c, :],
                            start=(c == 0),
                            stop=(c == NCH - 1),
                        )
                    # col 64 of the matmul output is s = sum_j e_j (the
                    # pad-column/selector trick); normalize via its recip
                    nc.vector.reciprocal(
                        out=rr[:, h : h + 1], in_=ops[:, h, H : H + 1]
                    )
                    nc.scalar.activation(
                        out=ob[:, 2 * (t % GRP) + h, :],
                        in_=ops[:, h, 0:H],
                        func=Copy,
                        scale=rr[:, h : h + 1],
                    )
                if t % GRP == GRP - 1:
                    g = t // GRP
                    r0 = g * GRP * TILE_ROWS
                    nc.sync.dma_start(
                        out=out_d[r0 : r0 + GRP * TILE_ROWS, :].rearrange(
                            "(p s) f -> p s f", p=128
                        ),
                        in_=st.pop(("ob", g)),
                    )
                    st.pop(("xb", g), None)

            do_load(0)
            do_load(1)
            stages = [
                (s_load, 0),
                (s_xbar, 0),
                (s_logits, 2),
                (s_att, 3),
                (s_erep, 3),
                (s_sp, 4),
                (s_final, 5),
            ]
            for r in range(NT + 5):
                for fn, off in stages:
                    tt = r - off
                    if 0 <= tt < NT:
                        fn(tt)

    nc.compile()
    return nc


def _fold_weights(Wa, ba, We, be, Ws, bs, Wo, bo):
    Wa = np.asarray(Wa, np.float64)
    We = np.asarray(We, np.float64)
    Ws = np.asarray(Ws, np.float64)
    Wo = np.asarray(Wo, np.float64)
    wa2 = Wa[SELF:, 0]  # [28]
    A_self = We[:SELF] @ Wo[:H] + (Ws[:SELF] + Ws[SELF:]) @ Wo[H:]  # [36, 64]
    A_pool = We[SELF:] @ Wo[:H]  # [28, 64]
    c = (
        np.asarray(be, np.float64) @ Wo[:H]
        + np.asarray(bs, np.float64) @ Wo[H:]
        + np.asarray(bo, np.float64)
    )  # [64]

    WLp = np.zeros((128, NCH, J + 1), np.float32)
    Bp = np.zeros((J, NCH, 128), np.float32)
    FWp = np.zeros((128, NCH, H + 2), np.float32)
    for ch in range(NCH):
        for p in range(128):
            f = 128 * ch + p
            if f >= OBS:
                continue
            if f < SELF:
                Bp[:, ch, p] = 1.0  # ones block -> s for self features
                FWp[p, ch, 0:H] = A_self[f]
            else:
                j0, k = divmod(f - SELF, OTH)
                WLp[p, ch, j0] = wa2[k]
                Bp[j0, ch, p] = 1.0
                FWp[p, ch, 0:H] = A_pool[k]
    # s-row: pad feature 568 (p=56, ch=4) is 1.0 on the host; all-ones
    # selector row makes erep there = s; indicator FW column 64 routes it
    # into the final matmul output
    Bp[:, 4, 56] = 1.0
    FWp[56, 4, H] = 1.0
    return WLp, Bp, FWp, c.astype(np.float32)


def kernel(x, Wa, ba, We, be, Ws, bs, Wo, bo):
    import ml_dtypes

    from concourse import bass_utils

    x = np.asarray(x, np.float32)
    assert x.shape == (BATCH, OBS), x.shape
    # host-side bf16 cast + pad to 640 (XBAR alignment): device loads
    # contiguous bf16 directly, no on-chip conversion
    xpad = np.zeros((BATCH, PADF), dtype=ml_dtypes.bfloat16)
    xpad[:, :OBS] = x.astype(ml_dtypes.bfloat16)
    xpad[:, OBS] = 1.0  # s-row source: xt[56, c=4] = 1 so erep there yields s

    WLp, Bp, FWp, c = _fold_weights(Wa, ba, We, be, Ws, bs, Wo, bo)

    if "nc" not in _CACHE:
        _CACHE["nc"] = _build_nc()
    nc = _CACHE["nc"]

    in_maps = []
    for i in range(NCORES):
        in_maps.append(
            {
                "x_in": xpad[i * ROWS_PER_CORE : (i + 1) * ROWS_PER_CORE],
                "wl_in": WLp,
                "bsel_in": Bp,
                "fw_in": FWp,
            }
        )

    res = bass_utils.run_bass_kernel_spmd(
        nc,
        in_maps,
        core_ids=list(range(NCORES)),
        trace=_CACHE.get("trace", False),
        **_CACHE.get("run_kwargs", {}),
    )
    _CACHE["last_results"] = res

    out = np.concatenate([np.asarray(res.results[i]["out"]) for i in range(NCORES)], 0)
    if np.any(c):
        out = out + c[None, :]
    return out.astype(np.float32)



# revision 4
# speedup vs baseline: 1.0137x; 1.0137x over previous
"""Trainium2 Bass kernel for nn_AlwGAT (GAT-style message passing), v3.

Math (equivalent to the reference):
  self = x[:, :36]; others = x[:, 36:].reshape(B, 19, 28)
  e_j  = exp(others_j . Wa[36:])        # softmax shift-invariance: self part cancels
  s    = sum_j e_j
  out  = [ self @ A_self + (sum_j (e_j/s) * others_j) @ A_pool ] + c
where
  A_self = We[:36] @ Wo[:64] + (Ws[:36] + Ws[36:]) @ Wo[64:]
  A_pool = We[36:] @ Wo[:64]
  c      = be @ Wo[:64] + bs @ Wo[64:] + bo     (added on host)

Dataflow (feature-major; host pre-transposes x per core to xT[568, 8192] bf16,
feature order = [others(532), self(36)]):
  per 512-row group (16 groups/core):
    xt    : 5 feature chunks on partitions (c0-3: 128@0, c4: 56@64), loaded by
            straight contiguous DMA (no on-chip transpose needed)
    logits: lt[128, 512] = sum_c WL_c^T @ xt_c   (5 accumulating MMs; WL has the
            19 logit columns replicated into all four 32-row groups)
    exp   : ACT -> eT[128, 512] bf16 (replicated copies at partitions 32g+j)
    erep  : 4 row-packed concurrent MMs (tile_position=(32g,0)) broadcast e_j to
            feature partitions via 0/1 selector B_g; 5th MM (B4) covers chunk 4
            and puts s = sum_j e_j on partitions 0..63 (ones columns)
    rr    : DVE reciprocal of er4[0:64] -> rr[64, 512]
    sp    : DVE xt * er (e-weighted features; self features get *s)
    final : outP[64, 512] = sum_c FW_c^T @ sp_c  (5 accumulating MMs, N=512)
    norm  : DVE outP * rr -> bf16 staging, coalesced DMA out per 4 groups
  out = s*(true_out - c), so out*rr + c on host recovers the reference.
"""

import os
import sys

if "/opt/trn_rl_repo" not in sys.path:
    sys.path.insert(0, "/opt/trn_rl_repo")

import numpy as np

SELF = 36
OTH = 28
J = 19
H = 64
OBS = SELF + OTH * J  # 568
NOTH = OTH * J  # 532
NCORES = 8
BATCH = 65536
ROWS_PER_CORE = BATCH // NCORES  # 8192
R = 512  # rows per compute group
NG = ROWS_PER_CORE // R  # 16
LG = 4  # compute groups per DMA load group
NL = NG // LG  # 4
RL = R * LG  # 2048 rows per load
C4P = 64  # chunk-4 partition base (others at 64..83, self at 84..119)
C4N = 20 + SELF  # 56 live partitions in chunk 4

_CACHE = {}


def _build_nc():
    import concourse.bass as bass  # noqa: F401
    import concourse.tile as tile
    from concourse import bacc, mybir

    f32 = mybir.dt.float32
    bf16 = mybir.dt.bfloat16

    nc = bacc.Bacc("TRN2", debug=False)
    x_d = nc.dram_tensor("x_in", [OBS, ROWS_PER_CORE], bf16, kind="ExternalInput").ap()
    wl_d = nc.dram_tensor("wl_in", [128, 5, 128], f32, kind="ExternalInput").ap()
    b_d = nc.dram_tensor("b_in", [128, 2, 128], f32, kind="ExternalInput").ap()
    fw_d = nc.dram_tensor("fw_in", [128, 5, H], f32, kind="ExternalInput").ap()
    out_d = nc.dram_tensor("out", [H, ROWS_PER_CORE], bf16, kind="ExternalOutput").ap()

    Exp = mybir.ActivationFunctionType.Exp

    with tile.TileContext(nc) as tc:
        with (
            tc.tile_pool(name="consts", bufs=1) as consts,
            tc.tile_pool(name="xt", bufs=2) as xt_pool,
            tc.tile_pool(name="eT", bufs=2) as eT_pool,
            tc.tile_pool(name="rr", bufs=2) as rr_pool,
            tc.tile_pool(name="sp", bufs=2) as sp_pool,
            tc.tile_pool(name="osb", bufs=2) as osb_pool,
            tc.tile_pool(name="psLT", bufs=1, space="PSUM") as lt_pool,
            tc.tile_pool(name="psER", bufs=1, space="PSUM") as er_pool,
            tc.tile_pool(name="psOUT", bufs=2, space="PSUM") as op_pool,
        ):
            # constants: stage f32, convert once to bf16
            wl_st = consts.tile([128, 5, 128], f32)
            nc.sync.dma_start(out=wl_st, in_=wl_d)
            wl_sb = consts.tile([128, 5, 128], bf16)
            nc.scalar.copy(out=wl_sb, in_=wl_st)
            b_st = consts.tile([128, 2, 128], f32)
            nc.sync.dma_start(out=b_st, in_=b_d)
            b_sb = consts.tile([128, 2, 128], bf16)
            nc.scalar.copy(out=b_sb, in_=b_st)
            fw_st = consts.tile([128, 5, H], f32)
            nc.sync.dma_start(out=fw_st, in_=fw_d)
            fw_sb = consts.tile([128, 5, H], bf16)
            nc.scalar.copy(out=fw_sb, in_=fw_st)

            st = {}

            def do_load(ld):
                r0 = ld * RL
                xb = xt_pool.tile([128, 5, RL], bf16, tag="xt")
                for c in range(4):
                    nc.sync.dma_start(
                        out=xb[:, c, :], in_=x_d[128 * c : 128 * (c + 1), r0 : r0 + RL]
                    )
                nc.sync.dma_start(
                    out=xb[C4P : C4P + C4N, 4, :], in_=x_d[512:OBS, r0 : r0 + RL]
                )
                st[("xb", ld)] = xb
                ob = osb_pool.tile([H, LG, R], bf16, tag="osb")
                st[("ob", ld)] = ob

            def s_group(t):
                ld = t // LG
                xb = st[("xb", ld)]
                sl = slice(R * (t % LG), R * (t % LG) + R)

                lt = lt_pool.tile([128, R], f32, tag="lt")
                for c in range(4):
                    nc.tensor.matmul(
                        lt, wl_sb[:, c, :], xb[:, c, sl], start=(c == 0), stop=False
                    )
                nc.tensor.matmul(
                    lt,
                    wl_sb[C4P : C4P + C4N, 4, :],
                    xb[C4P : C4P + C4N, 4, sl],
                    start=False,
                    stop=True,
                )

                eT = eT_pool.tile([128, R], bf16, tag="eT")
                nc.scalar.activation(out=eT, in_=lt, func=Exp)

                er = er_pool.tile([128, 5, R], f32, tag="er")
                for g in range(4):
                    nc.tensor.matmul(
                        er[:, g, :],
                        b_sb[32 * g : 32 * g + J, 0, :],
                        eT[32 * g : 32 * g + J, :],
                        start=True,
                        stop=True,
                        tile_position=(32 * g, 0),
                    )
                nc.tensor.matmul(
                    er[:, 4, :], b_sb[0:J, 1, :], eT[0:J, :], start=True, stop=True
                )

                rr = rr_pool.tile([H, R], f32, tag="rr")
                nc.vector.reciprocal(out=rr, in_=er[0:H, 4, :])

                sp = sp_pool.tile([128, 5, R], bf16, tag="sp")
                for c in range(4):
                    nc.vector.tensor_mul(sp[:, c, :], xb[:, c, sl], er[:, c, :])
                nc.vector.tensor_mul(
                    sp[C4P : C4P + C4N, 4, :],
                    xb[C4P : C4P + C4N, 4, sl],
                    er[C4P : C4P + C4N, 4, :],
                )

                ops = op_pool.tile([H, R], f32, tag="ops")
                for c in range(4):
                    nc.tensor.matmul(
                        ops, fw_sb[:, c, :], sp[:, c, :], start=(c == 0), stop=False
                    )
                nc.tensor.matmul(
                    ops,
                    fw_sb[C4P : C4P + C4N, 4, :],
                    sp[C4P : C4P + C4N, 4, :],
                    start=False,
                    stop=True,
                )

                ob = st[("ob", ld)]
                nc.vector.tensor_mul(ob[:, t % LG, :], ops, rr)

                if t % LG == LG - 1:
                    r0 = ld * RL
                    nc.scalar.dma_start(
                        out=out_d[:, r0 : r0 + RL], in_=st.pop(("ob", ld))
                    )
                    st.pop(("xb", ld), None)

            do_load(0)
            do_load(1)
            for t in range(NG):
                if t % LG == 0 and t // LG + 2 < NL:
                    do_load(t // LG + 2)
                s_group(t)

    nc.compile()
    return nc


def _fold_weights(Wa, ba, We, be, Ws, bs, Wo, bo):
    Wa = np.asarray(Wa, np.float64)
    We = np.asarray(We, np.float64)
    Ws = np.asarray(Ws, np.float64)
    Wo = np.asarray(Wo, np.float64)
    wa2 = Wa[SELF:, 0]  # [28]
    A_self = We[:SELF] @ Wo[:H] + (Ws[:SELF] + Ws[SELF:]) @ Wo[H:]  # [36, 64]
    A_pool = We[SELF:] @ Wo[:H]  # [28, 64]
    c = (
        np.asarray(be, np.float64) @ Wo[:H]
        + np.asarray(bs, np.float64) @ Wo[H:]
        + np.asarray(bo, np.float64)
    )  # [64]

    # feature-major order: f_or = 28*j + k for others, then 36 self features
    WLp = np.zeros((128, 5, 128), np.float32)
    Bp = np.zeros((128, 2, 128), np.float32)
    FWp = np.zeros((128, 5, H), np.float32)
    for ch in range(4):
        for p in range(128):
            f_or = 128 * ch + p
            j, k = divmod(f_or, OTH)
            for g in range(4):
                WLp[p, ch, 32 * g + j] = wa2[k]
            FWp[p, ch, :] = A_pool[k]
    # selector B_g (slot 0): rows 32g+j, cols p -> 1 iff feature 128g+p maps to j
    for g in range(4):
        for p in range(128):
            j = (128 * g + p) // OTH
            Bp[32 * g + j, 0, p] = 1.0
    # chunk 4: partitions C4P..C4P+19 = others f_or 512..531, C4P+20.. = self
    for i in range(20):
        f_or = 512 + i
        j, k = divmod(f_or, OTH)
        p = C4P + i
        for g in range(4):
            WLp[p, 4, 32 * g + j] = wa2[k]
        Bp[j, 1, p] = 1.0
        FWp[p, 4, :] = A_pool[k]
    for t in range(SELF):
        p = C4P + 20 + t
        Bp[0:J, 1, p] = 1.0  # ones -> er4 = s on self partitions
        FWp[p, 4, :] = A_self[t]
    Bp[0:J, 1, 0:H] = 1.0  # ones cols 0..63 -> er4[0:64] = s (for reciprocal)
    return WLp, Bp, FWp, c.astype(np.float32)


def kernel(x, Wa, ba, We, be, Ws, bs, Wo, bo):
    import ml_dtypes

    from concourse import bass_utils

    x = np.asarray(x, np.float32)
    assert x.shape == (BATCH, OBS), x.shape
    # host-side: bf16 cast + per-core feature-major transpose
    # feature order: others (x[:, 36:]) then self (x[:, :36])
    xb = x.astype(ml_dtypes.bfloat16)
    xT = np.empty((OBS, BATCH), dtype=ml_dtypes.bfloat16)
    xT[0:NOTH] = xb[:, SELF:].T
    xT[NOTH:OBS] = xb[:, 0:SELF].T

    WLp, Bp, FWp, c = _fold_weights(Wa, ba, We, be, Ws, bs, Wo, bo)

    if "nc" not in _CACHE:
        _CACHE["nc"] = _build_nc()
    nc = _CACHE["nc"]

    in_maps = []
    for i in range(NCORES):
        in_maps.append(
            {
                "x_in": np.ascontiguousarray(
                    xT[:, i * ROWS_PER_CORE : (i + 1) * ROWS_PER_CORE]
                ),
                "wl_in": WLp,
                "b_in": Bp,
                "fw_in": FWp,
            }
        )

    res = bass_utils.run_bass_kernel_spmd(
        nc,
        in_maps,
        core_ids=list(range(NCORES)),
        trace=_CACHE.get("trace", False),
        **_CACHE.get("run_kwargs", {}),
    )
    _CACHE["last_results"] = res

    # out_d is [64, 8192] per core; transpose back and normalize ordering
    out = np.concatenate(
        [np.asarray(res.results[i]["out"]).astype(np.float32).T for i in range(NCORES)],
        0,
    )
    if np.any(c):
        out = out + c[None, :]
    return out.astype(np.float32)


# revision 7
# speedup vs baseline: 1.2418x; 1.2250x over previous
"""Trainium2 Bass kernel for nn_AlwGAT (GAT-style message passing), v4.

Math (equivalent to the reference):
  self = x[:, :36]; others = x[:, 36:].reshape(B, 19, 28)
  e_j  = exp(others_j . Wa[36:])        # softmax shift-invariance: self part cancels
  s    = sum_j e_j
  out  = [ self @ A_self + (sum_j (e_j/s) * others_j) @ A_pool ] + c
where
  A_self = We[:36] @ Wo[:64] + (Ws[:36] + Ws[36:]) @ Wo[H:]
  A_pool = We[36:] @ Wo[:64]
  c      = be @ Wo[:64] + bs @ Wo[H:] + bo      (added on host)

Dataflow (feature-major; host pre-transposes x per core to xT[569, 8192] bf16,
feature order = [others(532), self(36), ones(1)]):
  per 512-row group (16 groups/core):
    xt    : 5 feature chunks on partitions (c0-3: 128@0, c4: 57@0), contiguous DMA
    logits: lt[128, 512] = sum_c WL_c^T @ xt_c   (5 accumulating MMs; WL holds the
            19 logit columns replicated into all four 32-row groups)
    exp   : ACT -> eT[128, 512] bf16 (replicas at partitions 32g+j)
    erep  : 4 row-packed concurrent MMs (tile_position=(32g,0)) broadcast e_j to
            feature partitions via 0/1 selector B_g; 5th MM (B4) covers chunk 4:
            others-indicators, ones for self (-> s), ones for the s-column (p56)
    sp    : ONE merged DVE mul [128, 5, 512]: xt * er  (self features get *s,
            the ones-feature becomes s itself)
    final : ops[65, 512] = sum_c FW_c^T @ sp_c  (5 accumulating MMs, N=512;
            col 64 routes the s feature -> ops[64] = s)
    copy  : ACT Copy ops -> bf16 staging; coalesced DMA out per 4 groups
  Device output is s*(true_out - c) stacked with s; host divides and adds c.
"""

import os
import sys

if "/opt/trn_rl_repo" not in sys.path:
    sys.path.insert(0, "/opt/trn_rl_repo")

import numpy as np

SELF = 36
OTH = 28
J = 19
H = 64
H1 = H + 1  # 64 outputs + s column
OBS = SELF + OTH * J  # 568
NOTH = OTH * J  # 532
XR = OBS + 1  # 569 device feature rows (ones row appended)
NCORES = 8
BATCH = 65536
ROWS_PER_CORE = BATCH // NCORES  # 8192
R = 512  # rows per compute group
NG = ROWS_PER_CORE // R  # 16
LG = 4  # compute groups per DMA load group
NL = NG // LG  # 4
RL = R * LG  # 2048 rows per load
C4N = 57  # live partitions in chunk 4 (20 others + 36 self + 1 ones)

_CACHE = {}


def _build_nc():
    import concourse.bass as bass  # noqa: F401
    import concourse.tile as tile
    from concourse import bacc, mybir

    f32 = mybir.dt.float32
    bf16 = mybir.dt.bfloat16

    nc = bacc.Bacc("TRN2", debug=False)
    x_d = nc.dram_tensor("x_in", [XR, ROWS_PER_CORE], bf16, kind="ExternalInput").ap()
    wl_d = nc.dram_tensor("wl_in", [128, 5, 128], f32, kind="ExternalInput").ap()
    b_d = nc.dram_tensor("b_in", [128, 2, 128], f32, kind="ExternalInput").ap()
    fw_d = nc.dram_tensor("fw_in", [128, 5, H1], f32, kind="ExternalInput").ap()
    out_d = nc.dram_tensor("out", [H1, ROWS_PER_CORE], bf16, kind="ExternalOutput").ap()

    Exp = mybir.ActivationFunctionType.Exp
    Copy = mybir.ActivationFunctionType.Copy

    with tile.TileContext(nc) as tc:
        with (
            tc.tile_pool(name="consts", bufs=1) as consts,
            tc.tile_pool(name="xt", bufs=2) as xt_pool,
            tc.tile_pool(name="eT", bufs=2) as eT_pool,
            tc.tile_pool(name="sp", bufs=2) as sp_pool,
            tc.tile_pool(name="osb", bufs=2) as osb_pool,
            tc.tile_pool(name="psLT", bufs=1, space="PSUM") as lt_pool,
            tc.tile_pool(name="psER", bufs=1, space="PSUM") as er_pool,
            tc.tile_pool(name="psOUT", bufs=2, space="PSUM") as op_pool,
        ):
            # constants: stage f32, convert once to bf16
            wl_st = consts.tile([128, 5, 128], f32)
            nc.sync.dma_start(out=wl_st, in_=wl_d)
            wl_sb = consts.tile([128, 5, 128], bf16)
            nc.scalar.copy(out=wl_sb, in_=wl_st)
            b_st = consts.tile([128, 2, 128], f32)
            nc.sync.dma_start(out=b_st, in_=b_d)
            b_sb = consts.tile([128, 2, 128], bf16)
            nc.scalar.copy(out=b_sb, in_=b_st)
            fw_st = consts.tile([128, 5, H1], f32)
            nc.sync.dma_start(out=fw_st, in_=fw_d)
            fw_sb = consts.tile([128, 5, H1], bf16)
            nc.scalar.copy(out=fw_sb, in_=fw_st)

            st = {}

            def do_load(ld):
                r0 = ld * RL
                xb = xt_pool.tile([128, 5, RL], bf16, tag="xt")
                for c in range(4):
                    nc.sync.dma_start(
                        out=xb[:, c, :], in_=x_d[128 * c : 128 * (c + 1), r0 : r0 + RL]
                    )
                nc.sync.dma_start(out=xb[0:C4N, 4, :], in_=x_d[512:XR, r0 : r0 + RL])
                # partitions C4N..127 of chunk 4 stay uninitialized; the merged
                # DVE mul multiplies them by er=0 and nothing reads the result
                st[("xb", ld)] = xb
                ob = osb_pool.tile([H1, LG, R], bf16, tag="osb")
                st[("ob", ld)] = ob

            def s_group(t):
                ld = t // LG
                xb = st[("xb", ld)]
                sl = slice(R * (t % LG), R * (t % LG) + R)

                lt = lt_pool.tile([128, R], f32, tag="lt")
                for c in range(4):
                    nc.tensor.matmul(
                        lt, wl_sb[:, c, :], xb[:, c, sl], start=(c == 0), stop=False
                    )
                nc.tensor.matmul(
                    lt,
                    wl_sb[0:C4N, 4, :],
                    xb[0:C4N, 4, sl],
                    start=False,
                    stop=True,
                )

                eT = eT_pool.tile([128, R], bf16, tag="eT")
                nc.scalar.activation(out=eT, in_=lt, func=Exp)

                er = er_pool.tile([128, 5, R], f32, tag="er")
                for g in range(4):
                    nc.tensor.matmul(
                        er[:, g, :],
                        b_sb[32 * g : 32 * g + J, 0, :],
                        eT[32 * g : 32 * g + J, :],
                        start=True,
                        stop=True,
                        tile_position=(32 * g, 0),
                    )
                nc.tensor.matmul(
                    er[:, 4, :], b_sb[0:J, 1, :], eT[0:J, :], start=True, stop=True
                )

                sp = sp_pool.tile([128, 5, R], bf16, tag="sp")
                nc.vector.tensor_mul(sp, xb[:, 0:5, sl], er)

                ops = op_pool.tile([H1, R], f32, tag="ops")
                for c in range(4):
                    nc.tensor.matmul(
                        ops, fw_sb[:, c, :], sp[:, c, :], start=(c == 0), stop=False
                    )
                nc.tensor.matmul(
                    ops,
                    fw_sb[0:C4N, 4, :],
                    sp[0:C4N, 4, :],
                    start=False,
                    stop=True,
                )

                ob = st[("ob", ld)]
                nc.scalar.activation(out=ob[:, t % LG, :], in_=ops, func=Copy)

                if t % LG == LG - 1:
                    r0 = ld * RL
                    nc.scalar.dma_start(
                        out=out_d[:, r0 : r0 + RL], in_=st.pop(("ob", ld))
                    )
                    st.pop(("xb", ld), None)

            do_load(0)
            do_load(1)
            for t in range(NG):
                if t % LG == 0 and t // LG + 2 < NL:
                    do_load(t // LG + 2)
                s_group(t)

    nc.compile()
    return nc


def _fold_weights(Wa, ba, We, be, Ws, bs, Wo, bo):
    Wa = np.asarray(Wa, np.float64)
    We = np.asarray(We, np.float64)
    Ws = np.asarray(Ws, np.float64)
    Wo = np.asarray(Wo, np.float64)
    wa2 = Wa[SELF:, 0]  # [28]
    A_self = We[:SELF] @ Wo[:H] + (Ws[:SELF] + Ws[SELF:]) @ Wo[H:]  # [36, 64]
    A_pool = We[SELF:] @ Wo[:H]  # [28, 64]
    c = (
        np.asarray(be, np.float64) @ Wo[:H]
        + np.asarray(bs, np.float64) @ Wo[H:]
        + np.asarray(bo, np.float64)
    )  # [64]

    # feature-major order: f_or = 28*j + k for others, then self, then ones
    WLp = np.zeros((128, 5, 128), np.float32)
    Bp = np.zeros((128, 2, 128), np.float32)
    FWp = np.zeros((128, 5, H1), np.float32)
    for ch in range(4):
        for p in range(128):
            f_or = 128 * ch + p
            j, k = divmod(f_or, OTH)
            for g in range(4):
                WLp[p, ch, 32 * g + j] = wa2[k]
            FWp[p, ch, 0:H] = A_pool[k]
    for g in range(4):
        for p in range(128):
            j = (128 * g + p) // OTH
            Bp[32 * g + j, 0, p] = 1.0
    # chunk 4: partitions 0..19 = others f_or 512..531, 20..55 = self, 56 = ones
    for i in range(20):
        f_or = 512 + i
        j, k = divmod(f_or, OTH)
        for g in range(4):
            WLp[i, 4, 32 * g + j] = wa2[k]
        Bp[j, 1, i] = 1.0
        FWp[i, 4, 0:H] = A_pool[k]
    for t in range(SELF):
        p = 20 + t
        Bp[0:J, 1, p] = 1.0  # ones -> er4 = s on self partitions
        FWp[p, 4, 0:H] = A_self[t]
    Bp[0:J, 1, 56] = 1.0  # ones -> er4[56] = s; x ones-row makes sp[56] = s
    FWp[56, 4, H] = 1.0  # route s into ops[64]
    return WLp, Bp, FWp, c.astype(np.float32)


def kernel(x, Wa, ba, We, be, Ws, bs, Wo, bo):
    import ml_dtypes

    from concourse import bass_utils

    x = np.asarray(x, np.float32)
    assert x.shape == (BATCH, OBS), x.shape
    # host-side: bf16 cast + per-core feature-major transpose
    # feature order: others (x[:, 36:]) then self (x[:, :36]) then ones
    xb = x.astype(ml_dtypes.bfloat16)
    xT = np.empty((XR, BATCH), dtype=ml_dtypes.bfloat16)
    xT[0:NOTH] = xb[:, SELF:].T
    xT[NOTH:OBS] = xb[:, 0:SELF].T
    xT[OBS] = 1.0

    WLp, Bp, FWp, c = _fold_weights(Wa, ba, We, be, Ws, bs, Wo, bo)

    if "nc" not in _CACHE:
        _CACHE["nc"] = _build_nc()
    nc = _CACHE["nc"]

    in_maps = []
    for i in range(NCORES):
        in_maps.append(
            {
                "x_in": np.ascontiguousarray(
                    xT[:, i * ROWS_PER_CORE : (i + 1) * ROWS_PER_CORE]
                ),
                "wl_in": WLp,
                "b_in": Bp,
                "fw_in": FWp,
            }
        )

    res = bass_utils.run_bass_kernel_spmd(
        nc,
        in_maps,
        core_ids=list(range(NCORES)),
        trace=_CACHE.get("trace", False),
        **_CACHE.get("run_kwargs", {}),
    )
    _CACHE["last_results"] = res

    # out_d is [65, 8192] per core: rows 0..63 = s*(out-c), row 64 = s
    out = np.concatenate(
        [np.asarray(res.results[i]["out"]).astype(np.float32).T for i in range(NCORES)],
        0,
    )
    out = out[:, 0:H] / out[:, H : H + 1]
    out = out + c[None, :]
    return out.astype(np.float32)


# revision 9
# speedup vs baseline: 1.4436x; 1.1625x over previous
"""Trainium2 Bass kernel for nn_AlwGAT (GAT-style message passing), v4.

Math (equivalent to the reference):
  self = x[:, :36]; others = x[:, 36:].reshape(B, 19, 28)
  e_j  = exp(others_j . Wa[36:])        # softmax shift-invariance: self part cancels
  s    = sum_j e_j
  out  = [ self @ A_self + (sum_j (e_j/s) * others_j) @ A_pool ] + c
where
  A_self = We[:36] @ Wo[:64] + (Ws[:36] + Ws[36:]) @ Wo[H:]
  A_pool = We[36:] @ Wo[:64]
  c      = be @ Wo[:64] + bs @ Wo[H:] + bo      (added on host)

Dataflow (feature-major; host pre-transposes x per core to xT[569, 8192] bf16,
feature order = [others(532), self(36), ones(1)]):
  per 512-row group (16 groups/core):
    xt    : 5 feature chunks on partitions (c0-3: 128@0, c4: 57@0), contiguous DMA
    logits: lt[128, 512] = sum_c WL_c^T @ xt_c   (5 accumulating MMs; WL holds the
            19 logit columns replicated into all four 32-row groups)
    exp   : ACT -> eT[128, 512] bf16 (replicas at partitions 32g+j)
    erep  : 4 row-packed concurrent MMs (tile_position=(32g,0)) broadcast e_j to
            feature partitions via 0/1 selector B_g; 5th MM (B4) covers chunk 4:
            others-indicators, ones for self (-> s), ones for the s-column (p56)
    sp    : ONE merged DVE mul [128, 5, 512]: xt * er  (self features get *s,
            the ones-feature becomes s itself)
    final : ops[65, 512] = sum_c FW_c^T @ sp_c  (5 accumulating MMs, N=512;
            col 64 routes the s feature -> ops[64] = s)
    copy  : ACT Copy ops -> bf16 staging; coalesced DMA out per 4 groups
  Device output is s*(true_out - c) stacked with s; host divides and adds c.
"""

import os
import sys

if "/opt/trn_rl_repo" not in sys.path:
    sys.path.insert(0, "/opt/trn_rl_repo")

import numpy as np

SELF = 36
OTH = 28
J = 19
H = 64
H1 = H + 1  # 64 outputs + s column
OBS = SELF + OTH * J  # 568
NOTH = OTH * J  # 532
XR = OBS + 1  # 569 device feature rows (ones row appended)
NCORES = 8
BATCH = 65536
ROWS_PER_CORE = BATCH // NCORES  # 8192
R = 512  # rows per compute group
NG = ROWS_PER_CORE // R  # 16
LG = 4  # compute groups per DMA load group
NL = NG // LG  # 4
RL = R * LG  # 2048 rows per load
C4N = 57  # live partitions in chunk 4 (20 others + 36 self + 1 ones)

_CACHE = {}


def _build_nc():
    import concourse.bass as bass  # noqa: F401
    import concourse.tile as tile
    from concourse import bacc, mybir

    f32 = mybir.dt.float32
    bf16 = mybir.dt.bfloat16

    nc = bacc.Bacc("TRN2", debug=False)
    x_d = nc.dram_tensor("x_in", [XR, ROWS_PER_CORE], bf16, kind="ExternalInput").ap()
    wl_d = nc.dram_tensor("wl_in", [128, 5, 128], f32, kind="ExternalInput").ap()
    b_d = nc.dram_tensor("b_in", [128, 2, 128], f32, kind="ExternalInput").ap()
    fw_d = nc.dram_tensor("fw_in", [128, 5, H1], f32, kind="ExternalInput").ap()
    out_d = nc.dram_tensor("out", [H1, ROWS_PER_CORE], bf16, kind="ExternalOutput").ap()

    Exp = mybir.ActivationFunctionType.Exp
    Copy = mybir.ActivationFunctionType.Copy

    with tile.TileContext(nc) as tc:
        with (
            tc.tile_pool(name="consts", bufs=1) as consts,
            tc.tile_pool(name="xt", bufs=2) as xt_pool,
            tc.tile_pool(name="eT", bufs=2) as eT_pool,
            tc.tile_pool(name="sp", bufs=3) as sp_pool,
            tc.tile_pool(name="osb", bufs=2) as osb_pool,
            tc.tile_pool(name="psLT", bufs=1, space="PSUM") as lt_pool,
            tc.tile_pool(name="psER", bufs=1, space="PSUM") as er_pool,
            tc.tile_pool(name="psOUT", bufs=2, space="PSUM") as op_pool,
        ):
            # constants: stage f32, convert once to bf16
            wl_st = consts.tile([128, 5, 128], f32)
            nc.sync.dma_start(out=wl_st, in_=wl_d)
            wl_sb = consts.tile([128, 5, 128], bf16)
            nc.scalar.copy(out=wl_sb, in_=wl_st)
            b_st = consts.tile([128, 2, 128], f32)
            nc.sync.dma_start(out=b_st, in_=b_d)
            b_sb = consts.tile([128, 2, 128], bf16)
            nc.scalar.copy(out=b_sb, in_=b_st)
            fw_st = consts.tile([128, 5, H1], f32)
            nc.sync.dma_start(out=fw_st, in_=fw_d)
            fw_sb = consts.tile([128, 5, H1], bf16)
            nc.scalar.copy(out=fw_sb, in_=fw_st)

            st = {}

            def do_load(ld):
                r0 = ld * RL
                xb = xt_pool.tile([128, 5, RL], bf16, tag="xt")
                for c in range(4):
                    nc.sync.dma_start(
                        out=xb[:, c, :], in_=x_d[128 * c : 128 * (c + 1), r0 : r0 + RL]
                    )
                nc.sync.dma_start(out=xb[0:C4N, 4, :], in_=x_d[512:XR, r0 : r0 + RL])
                # partitions C4N..127 of chunk 4 stay uninitialized; the merged
                # DVE mul multiplies them by er=0 and nothing reads the result
                st[("xb", ld)] = xb
                ob = osb_pool.tile([H1, LG, R], bf16, tag="osb")
                st[("ob", ld)] = ob

            def s_load(t):
                if t % LG == 0 and t // LG + 2 < NL:
                    do_load(t // LG + 2)

            def s_logits(t):
                xb = st[("xb", t // LG)]
                sl = slice(R * (t % LG), R * (t % LG) + R)
                lt = lt_pool.tile([128, R], f32, tag="lt")
                for c in range(4):
                    nc.tensor.matmul(
                        lt, wl_sb[:, c, :], xb[:, c, sl], start=(c == 0), stop=False
                    )
                nc.tensor.matmul(
                    lt, wl_sb[0:C4N, 4, :], xb[0:C4N, 4, sl], start=False, stop=True
                )
                st[("lt", t)] = lt

            def s_exp(t):
                lt = st.pop(("lt", t))
                eT = eT_pool.tile([128, R], bf16, tag="eT")
                nc.scalar.activation(out=eT, in_=lt, func=Exp)
                st[("eT", t)] = eT

            def s_final(t):
                # final MMs for group t, emitted one iteration later so the PE
                # never head-of-line blocks on the DVE sp of the same group
                sp = st.pop(("sp", t))
                ops = op_pool.tile([H1, R], f32, tag="ops")
                for c in range(4):
                    nc.tensor.matmul(
                        ops, fw_sb[:, c, :], sp[:, c, :], start=(c == 0), stop=False
                    )
                nc.tensor.matmul(
                    ops, fw_sb[0:C4N, 4, :], sp[0:C4N, 4, :], start=False, stop=True
                )
                st[("ops", t)] = ops

            def s_erep(t):
                eT = st.pop(("eT", t))
                er = er_pool.tile([128, 5, R], f32, tag="er")
                for g in range(4):
                    nc.tensor.matmul(
                        er[:, g, :],
                        b_sb[32 * g : 32 * g + J, 0, :],
                        eT[32 * g : 32 * g + J, :],
                        start=True,
                        stop=True,
                        tile_position=(32 * g, 0),
                    )
                nc.tensor.matmul(
                    er[:, 4, :], b_sb[0:J, 1, :], eT[0:J, :], start=True, stop=True
                )
                st[("er", t)] = er

            def s_sp(t):
                xb = st[("xb", t // LG)]
                sl = slice(R * (t % LG), R * (t % LG) + R)
                er = st.pop(("er", t))
                sp = sp_pool.tile([128, 5, R], bf16, tag="sp")
                nc.vector.tensor_mul(sp, xb[:, 0:5, sl], er)
                st[("sp", t)] = sp

            def s_copy(t):
                ops = st.pop(("ops", t))
                ob = st[("ob", t // LG)]
                nc.scalar.activation(out=ob[:, t % LG, :], in_=ops, func=Copy)

            def s_store(t):
                if t % LG == LG - 1:
                    ld = t // LG
                    r0 = ld * RL
                    nc.scalar.dma_start(
                        out=out_d[:, r0 : r0 + RL], in_=st.pop(("ob", ld))
                    )
                    st.pop(("xb", ld), None)

            do_load(0)
            do_load(1)
            stages = [
                (s_load, 0),
                (s_logits, 0),
                (s_exp, 0),
                (s_final, 2),
                (s_erep, 0),
                (s_sp, 0),
                (s_copy, 2),
                (s_store, 2),
            ]
            for r in range(NG + 2):
                for fn, off in stages:
                    tt = r - off
                    if 0 <= tt < NG:
                        fn(tt)

    nc.compile()
    return nc


def _fold_weights(Wa, ba, We, be, Ws, bs, Wo, bo):
    Wa = np.asarray(Wa, np.float64)
    We = np.asarray(We, np.float64)
    Ws = np.asarray(Ws, np.float64)
    Wo = np.asarray(Wo, np.float64)
    wa2 = Wa[SELF:, 0]  # [28]
    A_self = We[:SELF] @ Wo[:H] + (Ws[:SELF] + Ws[SELF:]) @ Wo[H:]  # [36, 64]
    A_pool = We[SELF:] @ Wo[:H]  # [28, 64]
    c = (
        np.asarray(be, np.float64) @ Wo[:H]
        + np.asarray(bs, np.float64) @ Wo[H:]
        + np.asarray(bo, np.float64)
    )  # [64]

    # feature-major order: f_or = 28*j + k for others, then self, then ones
    WLp = np.zeros((128, 5, 128), np.float32)
    Bp = np.zeros((128, 2, 128), np.float32)
    FWp = np.zeros((128, 5, H1), np.float32)
    for ch in range(4):
        for p in range(128):
            f_or = 128 * ch + p
            j, k = divmod(f_or, OTH)
            for g in range(4):
                WLp[p, ch, 32 * g + j] = wa2[k]
            FWp[p, ch, 0:H] = A_pool[k]
    for g in range(4):
        for p in range(128):
            j = (128 * g + p) // OTH
            Bp[32 * g + j, 0, p] = 1.0
    # chunk 4: partitions 0..19 = others f_or 512..531, 20..55 = self, 56 = ones
    for i in range(20):
        f_or = 512 + i
        j, k = divmod(f_or, OTH)
        for g in range(4):
            WLp[i, 4, 32 * g + j] = wa2[k]
        Bp[j, 1, i] = 1.0
        FWp[i, 4, 0:H] = A_pool[k]
    for t in range(SELF):
        p = 20 + t
        Bp[0:J, 1, p] = 1.0  # ones -> er4 = s on self partitions
        FWp[p, 4, 0:H] = A_self[t]
    Bp[0:J, 1, 56] = 1.0  # ones -> er4[56] = s; x ones-row makes sp[56] = s
    FWp[56, 4, H] = 1.0  # route s into ops[64]
    return WLp, Bp, FWp, c.astype(np.float32)


def kernel(x, Wa, ba, We, be, Ws, bs, Wo, bo):
    import ml_dtypes

    from concourse import bass_utils

    x = np.asarray(x, np.float32)
    assert x.shape == (BATCH, OBS), x.shape
    # host-side: bf16 cast + per-core feature-major transpose
    # feature order: others (x[:, 36:]) then self (x[:, :36]) then ones
    xb = x.astype(ml_dtypes.bfloat16)
    xT = np.empty((XR, BATCH), dtype=ml_dtypes.bfloat16)
    xT[0:NOTH] = xb[:, SELF:].T
    xT[NOTH:OBS] = xb[:, 0:SELF].T
    xT[OBS] = 1.0

    WLp, Bp, FWp, c = _fold_weights(Wa, ba, We, be, Ws, bs, Wo, bo)

    if "nc" not in _CACHE:
        _CACHE["nc"] = _build_nc()
    nc = _CACHE["nc"]

    in_maps = []
    for i in range(NCORES):
        in_maps.append(
            {
                "x_in": np.ascontiguousarray(
                    xT[:, i * ROWS_PER_CORE : (i + 1) * ROWS_PER_CORE]
                ),
                "wl_in": WLp,
                "b_in": Bp,
                "fw_in": FWp,
            }
        )

    res = bass_utils.run_bass_kernel_spmd(
        nc,
        in_maps,
        core_ids=list(range(NCORES)),
        trace=_CACHE.get("trace", False),
        **_CACHE.get("run_kwargs", {}),
    )
    _CACHE["last_results"] = res

    # out_d is [65, 8192] per core: rows 0..63 = s*(out-c), row 64 = s
    out = np.concatenate(
        [np.asarray(res.results[i]["out"]).astype(np.float32).T for i in range(NCORES)],
        0,
    )
    out = out[:, 0:H] / out[:, H : H + 1]
    out = out + c[None, :]
    return out.astype(np.float32)


# revision 10
# speedup vs baseline: 1.5829x; 1.0965x over previous
"""Trainium2 Bass kernel for nn_AlwGAT (GAT-style message passing), v4.

Math (equivalent to the reference):
  self = x[:, :36]; others = x[:, 36:].reshape(B, 19, 28)
  e_j  = exp(others_j . Wa[36:])        # softmax shift-invariance: self part cancels
  s    = sum_j e_j
  out  = [ self @ A_self + (sum_j (e_j/s) * others_j) @ A_pool ] + c
where
  A_self = We[:36] @ Wo[:64] + (Ws[:36] + Ws[36:]) @ Wo[H:]
  A_pool = We[36:] @ Wo[:64]
  c      = be @ Wo[:64] + bs @ Wo[H:] + bo      (added on host)

Dataflow (feature-major; host pre-transposes x per core to xT[569, 8192] bf16,
feature order = [others(532), self(36), ones(1)]):
  per 512-row group (16 groups/core):
    xt    : 5 feature chunks on partitions (c0-3: 128@0, c4: 57@0), contiguous DMA
    logits: lt[128, 512] = sum_c WL_c^T @ xt_c   (5 accumulating MMs; WL holds the
            19 logit columns replicated into all four 32-row groups)
    exp   : ACT -> eT[128, 512] bf16 (replicas at partitions 32g+j)
    erep  : 4 row-packed concurrent MMs (tile_position=(32g,0)) broadcast e_j to
            feature partitions via 0/1 selector B_g; 5th MM (B4) covers chunk 4:
            others-indicators, ones for self (-> s), ones for the s-column (p56)
    sp    : ONE merged DVE mul [128, 5, 512]: xt * er  (self features get *s,
            the ones-feature becomes s itself)
    final : ops[65, 512] = sum_c FW_c^T @ sp_c  (5 accumulating MMs, N=512;
            col 64 routes the s feature -> ops[64] = s)
    copy  : ACT Copy ops -> bf16 staging; coalesced DMA out per 4 groups
  Device output is s*(true_out - c) stacked with s; host divides and adds c.
"""

import os
import sys

if "/opt/trn_rl_repo" not in sys.path:
    sys.path.insert(0, "/opt/trn_rl_repo")

import numpy as np

SELF = 36
OTH = 28
J = 19
H = 64
H1 = H + 1  # 64 outputs + s column
OBS = SELF + OTH * J  # 568
NOTH = OTH * J  # 532
XR = OBS + 1  # 569 device feature rows (ones row appended)
NCORES = 8
BATCH = 65536
ROWS_PER_CORE = BATCH // NCORES  # 8192
R = 512  # rows per compute group
NG = ROWS_PER_CORE // R  # 16
LG = 4  # compute groups per DMA load group
NL = NG // LG  # 4
RL = R * LG  # 2048 rows per load
C4N = 57  # live partitions in chunk 4 (20 others + 36 self + 1 ones)

_CACHE = {}


def _build_nc():
    import concourse.bass as bass  # noqa: F401
    import concourse.tile as tile
    from concourse import bacc, mybir

    f32 = mybir.dt.float32
    bf16 = mybir.dt.bfloat16

    nc = bacc.Bacc("TRN2", debug=False)
    x_d = nc.dram_tensor("x_in", [XR, ROWS_PER_CORE], bf16, kind="ExternalInput").ap()
    wl_d = nc.dram_tensor("wl_in", [128, 5, 128], f32, kind="ExternalInput").ap()
    b_d = nc.dram_tensor("b_in", [128, 2, 128], f32, kind="ExternalInput").ap()
    fw_d = nc.dram_tensor("fw_in", [128, 5, H1], f32, kind="ExternalInput").ap()
    out_d = nc.dram_tensor("out", [H1, ROWS_PER_CORE], bf16, kind="ExternalOutput").ap()

    Exp = mybir.ActivationFunctionType.Exp
    Copy = mybir.ActivationFunctionType.Copy

    with tile.TileContext(nc) as tc:
        with (
            tc.tile_pool(name="consts", bufs=1) as consts,
            tc.tile_pool(name="xt", bufs=2) as xt_pool,
            tc.tile_pool(name="eT", bufs=2) as eT_pool,
            tc.tile_pool(name="sp", bufs=3) as sp_pool,
            tc.tile_pool(name="osb", bufs=2) as osb_pool,
            tc.tile_pool(name="psLT", bufs=2, space="PSUM") as lt_pool,
            tc.tile_pool(name="psER", bufs=1, space="PSUM") as er_pool,
            tc.tile_pool(name="psOUT", bufs=1, space="PSUM") as op_pool,
        ):
            # constants: stage f32, convert once to bf16
            wl_st = consts.tile([128, 5, 128], f32)
            nc.sync.dma_start(out=wl_st, in_=wl_d)
            wl_sb = consts.tile([128, 5, 128], bf16)
            nc.scalar.copy(out=wl_sb, in_=wl_st)
            b_st = consts.tile([128, 2, 128], f32)
            nc.sync.dma_start(out=b_st, in_=b_d)
            b_sb = consts.tile([128, 2, 128], bf16)
            nc.scalar.copy(out=b_sb, in_=b_st)
            fw_st = consts.tile([128, 5, H1], f32)
            nc.sync.dma_start(out=fw_st, in_=fw_d)
            fw_sb = consts.tile([128, 5, H1], bf16)
            nc.scalar.copy(out=fw_sb, in_=fw_st)

            st = {}

            def do_load(ld):
                r0 = ld * RL
                xb = xt_pool.tile([128, 5, RL], bf16, tag="xt")
                for c in range(4):
                    nc.sync.dma_start(
                        out=xb[:, c, :], in_=x_d[128 * c : 128 * (c + 1), r0 : r0 + RL]
                    )
                nc.sync.dma_start(out=xb[0:C4N, 4, :], in_=x_d[512:XR, r0 : r0 + RL])
                # partitions C4N..127 of chunk 4 stay uninitialized; the merged
                # DVE mul multiplies them by er=0 and nothing reads the result
                st[("xb", ld)] = xb
                ob = osb_pool.tile([H1, LG, R], bf16, tag="osb")
                st[("ob", ld)] = ob

            def s_load(t):
                if t % LG == 0 and t // LG + 2 < NL:
                    do_load(t // LG + 2)

            def s_logits(t):
                xb = st[("xb", t // LG)]
                sl = slice(R * (t % LG), R * (t % LG) + R)
                lt = lt_pool.tile([128, R], f32, tag="lt")
                for c in range(4):
                    nc.tensor.matmul(
                        lt, wl_sb[:, c, :], xb[:, c, sl], start=(c == 0), stop=False
                    )
                nc.tensor.matmul(
                    lt, wl_sb[0:C4N, 4, :], xb[0:C4N, 4, sl], start=False, stop=True
                )
                st[("lt", t)] = lt

            def s_exp(t):
                lt = st.pop(("lt", t))
                eT = eT_pool.tile([128, R], bf16, tag="eT")
                nc.scalar.activation(out=eT, in_=lt, func=Exp)
                st[("eT", t)] = eT

            def s_final(t):
                # final MMs for group t, emitted one iteration later so the PE
                # never head-of-line blocks on the DVE sp of the same group
                sp = st.pop(("sp", t))
                ops = op_pool.tile([H1, R], f32, tag="ops")
                for c in range(4):
                    nc.tensor.matmul(
                        ops, fw_sb[:, c, :], sp[:, c, :], start=(c == 0), stop=False
                    )
                nc.tensor.matmul(
                    ops, fw_sb[0:C4N, 4, :], sp[0:C4N, 4, :], start=False, stop=True
                )
                st[("ops", t)] = ops

            def s_erep(t):
                eT = st.pop(("eT", t))
                er = er_pool.tile([128, 5, R], f32, tag="er")
                for g in range(4):
                    nc.tensor.matmul(
                        er[:, g, :],
                        b_sb[32 * g : 32 * g + J, 0, :],
                        eT[32 * g : 32 * g + J, :],
                        start=True,
                        stop=True,
                        tile_position=(32 * g, 0),
                    )
                nc.tensor.matmul(
                    er[:, 4, :], b_sb[0:J, 1, :], eT[0:J, :], start=True, stop=True
                )
                st[("er", t)] = er

            def s_sp(t):
                xb = st[("xb", t // LG)]
                sl = slice(R * (t % LG), R * (t % LG) + R)
                er = st.pop(("er", t))
                sp = sp_pool.tile([128, 5, R], bf16, tag="sp")
                nc.vector.tensor_mul(sp, xb[:, 0:5, sl], er)
                st[("sp", t)] = sp

            def s_copy(t):
                ops = st.pop(("ops", t))
                ob = st[("ob", t // LG)]
                nc.scalar.activation(out=ob[:, t % LG, :], in_=ops, func=Copy)

            def s_store(t):
                if t % LG == LG - 1:
                    ld = t // LG
                    r0 = ld * RL
                    nc.scalar.dma_start(
                        out=out_d[:, r0 : r0 + RL], in_=st.pop(("ob", ld))
                    )
                    st.pop(("xb", ld), None)

            do_load(0)
            do_load(1)
            stages = [
                (s_load, 0),
                (s_logits, 0),
                (s_exp, 0),
                (s_final, 2),
                (s_erep, 0),
                (s_sp, 0),
                (s_copy, 2),
                (s_store, 2),
            ]
            for r in range(NG + 2):
                for fn, off in stages:
                    tt = r - off
                    if 0 <= tt < NG:
                        fn(tt)

    nc.compile()
    return nc


def _fold_weights(Wa, ba, We, be, Ws, bs, Wo, bo):
    Wa = np.asarray(Wa, np.float64)
    We = np.asarray(We, np.float64)
    Ws = np.asarray(Ws, np.float64)
    Wo = np.asarray(Wo, np.float64)
    wa2 = Wa[SELF:, 0]  # [28]
    A_self = We[:SELF] @ Wo[:H] + (Ws[:SELF] + Ws[SELF:]) @ Wo[H:]  # [36, 64]
    A_pool = We[SELF:] @ Wo[:H]  # [28, 64]
    c = (
        np.asarray(be, np.float64) @ Wo[:H]
        + np.asarray(bs, np.float64) @ Wo[H:]
        + np.asarray(bo, np.float64)
    )  # [64]

    # feature-major order: f_or = 28*j + k for others, then self, then ones
    WLp = np.zeros((128, 5, 128), np.float32)
    Bp = np.zeros((128, 2, 128), np.float32)
    FWp = np.zeros((128, 5, H1), np.float32)
    for ch in range(4):
        for p in range(128):
            f_or = 128 * ch + p
            j, k = divmod(f_or, OTH)
            for g in range(4):
                WLp[p, ch, 32 * g + j] = wa2[k]
            FWp[p, ch, 0:H] = A_pool[k]
    for g in range(4):
        for p in range(128):
            j = (128 * g + p) // OTH
            Bp[32 * g + j, 0, p] = 1.0
    # chunk 4: partitions 0..19 = others f_or 512..531, 20..55 = self, 56 = ones
    for i in range(20):
        f_or = 512 + i
        j, k = divmod(f_or, OTH)
        for g in range(4):
            WLp[i, 4, 32 * g + j] = wa2[k]
        Bp[j, 1, i] = 1.0
        FWp[i, 4, 0:H] = A_pool[k]
    for t in range(SELF):
        p = 20 + t
        Bp[0:J, 1, p] = 1.0  # ones -> er4 = s on self partitions
        FWp[p, 4, 0:H] = A_self[t]
    Bp[0:J, 1, 56] = 1.0  # ones -> er4[56] = s; x ones-row makes sp[56] = s
    FWp[56, 4, H] = 1.0  # route s into ops[64]
    return WLp, Bp, FWp, c.astype(np.float32)


def kernel(x, Wa, ba, We, be, Ws, bs, Wo, bo):
    import ml_dtypes

    from concourse import bass_utils

    x = np.asarray(x, np.float32)
    assert x.shape == (BATCH, OBS), x.shape
    # host-side: bf16 cast + per-core feature-major transpose
    # feature order: others (x[:, 36:]) then self (x[:, :36]) then ones
    xb = x.astype(ml_dtypes.bfloat16)
    xT = np.empty((XR, BATCH), dtype=ml_dtypes.bfloat16)
    xT[0:NOTH] = xb[:, SELF:].T
    xT[NOTH:OBS] = xb[:, 0:SELF].T
    xT[OBS] = 1.0

    WLp, Bp, FWp, c = _fold_weights(Wa, ba, We, be, Ws, bs, Wo, bo)

    if "nc" not in _CACHE:
        _CACHE["nc"] = _build_nc()
    nc = _CACHE["nc"]

    in_maps = []
    for i in range(NCORES):
        in_maps.append(
            {
                "x_in": np.ascontiguousarray(
                    xT[:, i * ROWS_PER_CORE : (i + 1) * ROWS_PER_CORE]
                ),
                "wl_in": WLp,
                "b_in": Bp,
                "fw_in": FWp,
            }
        )

    res = bass_utils.run_bass_kernel_spmd(
        nc,
        in_maps,
        core_ids=list(range(NCORES)),
        trace=_CACHE.get("trace", False),
        **_CACHE.get("run_kwargs", {}),
    )
    _CACHE["last_results"] = res

    # out_d is [65, 8192] per core: rows 0..63 = s*(out-c), row 64 = s
    out = np.concatenate(
        [np.asarray(res.results[i]["out"]).astype(np.float32).T for i in range(NCORES)],
        0,
    )
    out = out[:, 0:H] / out[:, H : H + 1]
    out = out + c[None, :]
    return out.astype(np.float32)


# revision 14
# speedup vs baseline: 1.7651x; 1.1151x over previous
"""Trainium2 Bass kernel for nn_AlwGAT (GAT-style message passing), v4.

Math (equivalent to the reference):
  self = x[:, :36]; others = x[:, 36:].reshape(B, 19, 28)
  e_j  = exp(others_j . Wa[36:])        # softmax shift-invariance: self part cancels
  s    = sum_j e_j
  out  = [ self @ A_self + (sum_j (e_j/s) * others_j) @ A_pool ] + c
where
  A_self = We[:36] @ Wo[:64] + (Ws[:36] + Ws[36:]) @ Wo[H:]
  A_pool = We[36:] @ Wo[:64]
  c      = be @ Wo[:64] + bs @ Wo[H:] + bo      (added on host)

Dataflow (feature-major; host pre-transposes x per core to xT[569, 8192] bf16,
feature order = [others(532), self(36), ones(1)]):
  per 512-row group (16 groups/core):
    xt    : 5 feature chunks on partitions (c0-3: 128@0, c4: 57@0), contiguous DMA
    logits: lt[128, 512] = sum_c WL_c^T @ xt_c   (5 accumulating MMs; WL holds the
            19 logit columns replicated into all four 32-row groups)
    exp   : ACT -> eT[128, 512] bf16 (replicas at partitions 32g+j)
    erep  : 4 row-packed concurrent MMs (tile_position=(32g,0)) broadcast e_j to
            feature partitions via 0/1 selector B_g; 5th MM (B4) covers chunk 4:
            others-indicators, ones for self (-> s), ones for the s-column (p56)
    sp    : ONE merged DVE mul [128, 5, 512]: xt * er  (self features get *s,
            the ones-feature becomes s itself)
    final : ops[65, 512] = sum_c FW_c^T @ sp_c  (5 accumulating MMs, N=512;
            col 64 routes the s feature -> ops[64] = s)
    copy  : ACT Copy ops -> bf16 staging; coalesced DMA out per 4 groups
  Device output is s*(true_out - c) stacked with s; host divides and adds c.
"""

import os
import sys

if "/opt/trn_rl_repo" not in sys.path:
    sys.path.insert(0, "/opt/trn_rl_repo")

import numpy as np

SELF = 36
OTH = 28
J = 19
H = 64
H1 = H + 1  # 64 outputs + s column
OBS = SELF + OTH * J  # 568
NOTH = OTH * J  # 532
XR = OBS + 1  # 569 device feature rows (ones row appended)
NCORES = 8
BATCH = 65536
ROWS_PER_CORE = BATCH // NCORES  # 8192
R = 512  # rows per compute group
NG = ROWS_PER_CORE // R  # 16
LG = 4  # compute groups per DMA load group
NL = NG // LG  # 4
RL = R * LG  # 2048 rows per load
C4N = 57  # live partitions in chunk 4 (20 others + 36 self + 1 ones)

_CACHE = {}


def _build_nc():
    import concourse.bass as bass  # noqa: F401
    import concourse.tile as tile
    from concourse import bacc, mybir

    f32 = mybir.dt.float32
    bf16 = mybir.dt.bfloat16

    nc = bacc.Bacc("TRN2", debug=False)
    x_d = nc.dram_tensor("x_in", [XR, ROWS_PER_CORE], bf16, kind="ExternalInput").ap()
    wl_d = nc.dram_tensor("wl_in", [128, 5, 128], f32, kind="ExternalInput").ap()
    b_d = nc.dram_tensor("b_in", [128, 2, 128], f32, kind="ExternalInput").ap()
    fw_d = nc.dram_tensor("fw_in", [128, 5, H1], f32, kind="ExternalInput").ap()
    out_d = nc.dram_tensor("out", [H1, ROWS_PER_CORE], bf16, kind="ExternalOutput").ap()

    Exp = mybir.ActivationFunctionType.Exp
    Copy = mybir.ActivationFunctionType.Copy

    with tile.TileContext(nc) as tc:
        with (
            tc.tile_pool(name="consts", bufs=1) as consts,
            tc.tile_pool(name="xt", bufs=4) as xt_pool,
            tc.tile_pool(name="eT", bufs=2) as eT_pool,
            tc.tile_pool(name="sp", bufs=3) as sp_pool,
            tc.tile_pool(name="osb", bufs=2) as osb_pool,
            tc.tile_pool(name="psLT", bufs=2, space="PSUM") as lt_pool,
            tc.tile_pool(name="psER", bufs=1, space="PSUM") as er_pool,
            tc.tile_pool(name="psOUT", bufs=1, space="PSUM") as op_pool,
        ):
            # constants: stage f32, convert once to bf16
            wl_st = consts.tile([128, 5, 128], f32)
            nc.sync.dma_start(out=wl_st, in_=wl_d)
            wl_sb = consts.tile([128, 5, 128], bf16)
            nc.scalar.copy(out=wl_sb, in_=wl_st)
            b_st = consts.tile([128, 2, 128], f32)
            nc.sync.dma_start(out=b_st, in_=b_d)
            b_sb = consts.tile([128, 2, 128], bf16)
            nc.scalar.copy(out=b_sb, in_=b_st)
            fw_st = consts.tile([128, 5, H1], f32)
            nc.sync.dma_start(out=fw_st, in_=fw_d)
            fw_sb = consts.tile([128, 5, H1], bf16)
            nc.scalar.copy(out=fw_sb, in_=fw_st)

            st = {}

            def do_load(ld):
                r0 = ld * RL
                xb = xt_pool.tile([128, 5, RL], bf16, tag="xt")
                for c in range(4):
                    q = nc.sync if c % 2 == 0 else nc.scalar
                    q.dma_start(
                        out=xb[:, c, :], in_=x_d[128 * c : 128 * (c + 1), r0 : r0 + RL]
                    )
                nc.sync.dma_start(out=xb[0:C4N, 4, :], in_=x_d[512:XR, r0 : r0 + RL])
                # partitions C4N..127 of chunk 4 stay uninitialized; the merged
                # DVE mul multiplies them by er=0 and nothing reads the result
                st[("xb", ld)] = xb
                ob = osb_pool.tile([H1, LG, R], bf16, tag="osb")
                st[("ob", ld)] = ob

            def s_logits(t):
                xb = st[("xb", t // LG)]
                sl = slice(R * (t % LG), R * (t % LG) + R)
                lt = lt_pool.tile([128, R], f32, tag="lt")
                for c in range(4):
                    nc.tensor.matmul(
                        lt, wl_sb[:, c, :], xb[:, c, sl], start=(c == 0), stop=False
                    )
                nc.tensor.matmul(
                    lt, wl_sb[0:C4N, 4, :], xb[0:C4N, 4, sl], start=False, stop=True
                )
                st[("lt", t)] = lt

            def s_exp(t):
                lt = st.pop(("lt", t))
                eT = eT_pool.tile([128, R], bf16, tag="eT")
                nc.scalar.activation(out=eT, in_=lt, func=Exp)
                st[("eT", t)] = eT

            def s_final(t):
                # final MMs for group t, emitted one iteration later so the PE
                # never head-of-line blocks on the DVE sp of the same group
                sp = st.pop(("sp", t))
                ops = op_pool.tile([H1, R], f32, tag="ops")
                for c in range(4):
                    nc.tensor.matmul(
                        ops, fw_sb[:, c, :], sp[:, c, :], start=(c == 0), stop=False
                    )
                nc.tensor.matmul(
                    ops, fw_sb[0:C4N, 4, :], sp[0:C4N, 4, :], start=False, stop=True
                )
                st[("ops", t)] = ops

            def s_erep(t):
                eT = st.pop(("eT", t))
                er = er_pool.tile([128, 5, R], f32, tag="er")
                for g in range(4):
                    nc.tensor.matmul(
                        er[:, g, :],
                        b_sb[32 * g : 32 * g + J, 0, :],
                        eT[32 * g : 32 * g + J, :],
                        start=True,
                        stop=True,
                        tile_position=(32 * g, 0),
                    )
                nc.tensor.matmul(
                    er[:, 4, :], b_sb[0:J, 1, :], eT[0:J, :], start=True, stop=True
                )
                st[("er", t)] = er

            def s_sp(t):
                xb = st[("xb", t // LG)]
                sl = slice(R * (t % LG), R * (t % LG) + R)
                er = st.pop(("er", t))
                sp = sp_pool.tile([128, 5, R], bf16, tag="sp")
                nc.vector.tensor_mul(sp, xb[:, 0:5, sl], er)
                st[("sp", t)] = sp

            def s_copy(t):
                ops = st.pop(("ops", t))
                ob = st[("ob", t // LG)]
                nc.scalar.activation(out=ob[:, t % LG, :], in_=ops, func=Copy)

            def s_store(t):
                if t % LG == LG - 1:
                    ld = t // LG
                    r0 = ld * RL
                    nc.gpsimd.dma_start(
                        out=out_d[:, r0 : r0 + RL], in_=st.pop(("ob", ld))
                    )
                    st.pop(("xb", ld), None)

            for ld in range(NL):
                do_load(ld)
            stages = [
                (s_logits, 0),
                (s_exp, 0),
                (s_final, 2),
                (s_erep, 0),
                (s_sp, 0),
                (s_copy, 2),
                (s_store, 2),
            ]
            for r in range(NG + 2):
                for fn, off in stages:
                    tt = r - off
                    if 0 <= tt < NG:
                        fn(tt)

    nc.compile()
    return nc


def _fold_weights(Wa, ba, We, be, Ws, bs, Wo, bo):
    Wa = np.asarray(Wa, np.float64)
    We = np.asarray(We, np.float64)
    Ws = np.asarray(Ws, np.float64)
    Wo = np.asarray(Wo, np.float64)
    wa2 = Wa[SELF:, 0]  # [28]
    A_self = We[:SELF] @ Wo[:H] + (Ws[:SELF] + Ws[SELF:]) @ Wo[H:]  # [36, 64]
    A_pool = We[SELF:] @ Wo[:H]  # [28, 64]
    c = (
        np.asarray(be, np.float64) @ Wo[:H]
        + np.asarray(bs, np.float64) @ Wo[H:]
        + np.asarray(bo, np.float64)
    )  # [64]

    # feature-major order: f_or = 28*j + k for others, then self, then ones
    WLp = np.zeros((128, 5, 128), np.float32)
    Bp = np.zeros((128, 2, 128), np.float32)
    FWp = np.zeros((128, 5, H1), np.float32)
    for ch in range(4):
        for p in range(128):
            f_or = 128 * ch + p
            j, k = divmod(f_or, OTH)
            for g in range(4):
                WLp[p, ch, 32 * g + j] = wa2[k]
            FWp[p, ch, 0:H] = A_pool[k]
    for g in range(4):
        for p in range(128):
            j = (128 * g + p) // OTH
            Bp[32 * g + j, 0, p] = 1.0
    # chunk 4: partitions 0..19 = others f_or 512..531, 20..55 = self, 56 = ones
    for i in range(20):
        f_or = 512 + i
        j, k = divmod(f_or, OTH)
        for g in range(4):
            WLp[i, 4, 32 * g + j] = wa2[k]
        Bp[j, 1, i] = 1.0
        FWp[i, 4, 0:H] = A_pool[k]
    for t in range(SELF):
        p = 20 + t
        Bp[0:J, 1, p] = 1.0  # ones -> er4 = s on self partitions
        FWp[p, 4, 0:H] = A_self[t]
    Bp[0:J, 1, 56] = 1.0  # ones -> er4[56] = s; x ones-row makes sp[56] = s
    FWp[56, 4, H] = 1.0  # route s into ops[64]
    return WLp, Bp, FWp, c.astype(np.float32)


def kernel(x, Wa, ba, We, be, Ws, bs, Wo, bo):
    import ml_dtypes

    from concourse import bass_utils

    x = np.asarray(x, np.float32)
    assert x.shape == (BATCH, OBS), x.shape
    # host-side: bf16 cast + per-core feature-major transpose
    # feature order: others (x[:, 36:]) then self (x[:, :36]) then ones
    xb = x.astype(ml_dtypes.bfloat16)
    xT = np.empty((XR, BATCH), dtype=ml_dtypes.bfloat16)
    xT[0:NOTH] = xb[:, SELF:].T
    xT[NOTH:OBS] = xb[:, 0:SELF].T
    xT[OBS] = 1.0

    WLp, Bp, FWp, c = _fold_weights(Wa, ba, We, be, Ws, bs, Wo, bo)

    if "nc" not in _CACHE:
        _CACHE["nc"] = _build_nc()
    nc = _CACHE["nc"]

    in_maps = []
    for i in range(NCORES):
        in_maps.append(
            {
                "x_in": np.ascontiguousarray(
                    xT[:, i * ROWS_PER_CORE : (i + 1) * ROWS_PER_CORE]
                ),
                "wl_in": WLp,
                "b_in": Bp,
                "fw_in": FWp,
            }
        )

    res = bass_utils.run_bass_kernel_spmd(
        nc,
        in_maps,
        core_ids=list(range(NCORES)),
        trace=_CACHE.get("trace", False),
        **_CACHE.get("run_kwargs", {}),
    )
    _CACHE["last_results"] = res

    # out_d is [65, 8192] per core: rows 0..63 = s*(out-c), row 64 = s
    out = np.concatenate(
        [np.asarray(res.results[i]["out"]).astype(np.float32).T for i in range(NCORES)],
        0,
    )
    out = out[:, 0:H] / out[:, H : H + 1]
    out = out + c[None, :]
    return out.astype(np.float32)
